# revision 14
# baseline (speedup 1.0000x reference)
"""Sparse MoE kernel: device router + host dispatch + device per-expert FFN.

Phase A (device, SPMD over 8 cores, token-sharded): each core computes the
fp32 router (logits -> top-2 renormalized dense gate) for its 1024-token
shard. All routing FLOPs are on device.

Host dispatch: membership is already encoded in the gate (gate[t,e] > 0 iff
expert e is in token t's top-2), so the host only gathers each expert's
token rows of x (data movement, no routing math) and pads to a fixed
capacity (default 2176; actual per-expert counts are 1973..2151; zero-pad
tokens carry gate 0 and contribute nothing; the FFN NEFF is rebuilt larger
at runtime if an expert ever exceeds the built capacity).

Phase B (device, SPMD, expert-parallel): core e runs the FFN on its C
gathered tokens in bf16 with fp32 accumulation and scales by its gate
column. Host scatter-adds the two expert contributions per token and adds
the gate-weighted b2 term.
"""

import sys

for _p in ("/opt/trn_rl_repo",):
    if _p not in sys.path:
        sys.path.insert(0, _p)

from contextlib import ExitStack

import ml_dtypes
import numpy as np

import concourse.bass as bass
import concourse.mybir as mybir
import concourse.tile as tile
from concourse.bass_utils import run_bass_kernel_spmd

BF16 = ml_dtypes.bfloat16
F32 = mybir.dt.float32
BF = mybir.dt.bfloat16
AF = mybir.ActivationFunctionType
ALU = mybir.AluOpType
AX = mybir.AxisListType

B, S, D, H, E = 4, 2048, 1024, 4096, 8
NTOK = B * S
GT = 512
KC = D // 128
MC = H // 128
NSUB = GT // 128
RTOK = NTOK // E          # tokens per core in phase A (1024)
RNG = RTOK // GT          # 2 groups
# Default per-expert token capacity in phase B. Actual per-expert top-2 loads
# on this problem's fixed inputs are 1973..2151 (mean 2048). If an expert ever
# receives more than the built capacity, the FFN NEFF is rebuilt larger at
# runtime (see _get_ffn).
DEFAULT_CAP = 2176

_CACHE: dict = {}


_SPLIT_SKIP: set = set()


def _split_multi_waits(nc: bass.Bass) -> None:
    """Walrus allows one sync-wait command per instruction; split extras
    onto same-engine InstNoOps (same mechanism Tile uses for drains)."""
    for blk in nc.m.functions[0].blocks:
        insts = blk.instructions
        idx = 0
        while idx < len(insts):
            i = insts[idx]
            si = i.sync_info
            if (
                si is not None
                and len(si.on_wait) >= 2
                and type(i).__name__ not in _SPLIT_SKIP
            ):
                waits = list(si.on_wait)
                for w in waits[:-1]:
                    nop = mybir.InstNoOp(
                        name=nc.get_next_instruction_name(),
                        sync_info=mybir.SyncInfo(on_wait=[w], on_update=[]),
                        bass_nofuse=True,
                        engine=i.engine,
                    )
                    insts.insert(idx, nop)
                    idx += 1
                si.on_wait = [waits[-1]]
            idx += 1


def _emit_gate(nc, tc, pools, pr, br_sb, gateo_slice):
    """Top-2 renormalized gate from fp32 logits psum pr [128, E].
    Returns the [128, E] gate tile (also DMA'd to gateo_slice)."""
    g_pool, s_pool = pools
    logits = g_pool.tile([128, E], F32, name="logits")
    nc.vector.tensor_add(logits[:], pr[:], br_sb[:])
    m1n = s_pool.tile([128, 1], F32, name="m1n")
    nc.vector.tensor_reduce(m1n[:], logits[:], axis=AX.X, op=ALU.max, negate=True)
    mask1 = g_pool.tile([128, E], F32, name="mask1")
    nc.vector.tensor_scalar(mask1[:], logits[:], m1n[:, 0:1], 0.0, ALU.add, ALU.is_ge)
    l2 = g_pool.tile([128, E], F32, name="l2")
    nc.vector.tensor_scalar(l2[:], mask1[:], -1.0e30, None, ALU.mult)
    nc.vector.tensor_add(l2[:], l2[:], logits[:])
    m2n = s_pool.tile([128, 1], F32, name="m2n")
    nc.vector.tensor_reduce(m2n[:], l2[:], axis=AX.X, op=ALU.max, negate=True)
    z = g_pool.tile([128, E], F32, name="z")
    nc.scalar.activation(z[:], logits[:], AF.Exp, bias=m1n[:, 0:1])
    mask2 = g_pool.tile([128, E], F32, name="mask2")
    nc.vector.tensor_scalar(mask2[:], logits[:], m2n[:, 0:1], 0.0, ALU.add, ALU.is_ge)
    zs = g_pool.tile([128, E], F32, name="zs")
    nc.vector.tensor_mul(zs[:], z[:], mask2[:])
    den = s_pool.tile([128, 1], F32, name="den")
    nc.vector.tensor_reduce(den[:], zs[:], axis=AX.X, op=ALU.add)
    rden = s_pool.tile([128, 1], F32, name="rden")
    nc.vector.reciprocal(rden[:], den[:])
    gate = g_pool.tile([128, E], F32, name="gatet")
    nc.vector.tensor_scalar(gate[:], zs[:], rden[:, 0:1], None, ALU.mult)
    nc.sync.dma_start(gateo_slice, gate[:])
    return gate


def _build_router_nc() -> bass.Bass:
    """Phase A: fp32 router over this core's RTOK-token shard."""
    nc = bass.Bass()
    xf = nc.declare_dram_parameter("xf", [KC, 128, RTOK], F32, isOutput=False)
    wr = nc.declare_dram_parameter("wr", [128, KC * E], F32, isOutput=False)
    brt = nc.declare_dram_parameter("brt", [128, E], F32, isOutput=False)
    gateo = nc.declare_dram_parameter("gateo", [RTOK, E], F32, isOutput=True)

    with ExitStack() as ctx:
        tc = ctx.enter_context(tile.TileContext(nc))
        cpool = ctx.enter_context(tc.tile_pool(name="const", bufs=1))
        wr_sb = cpool.tile([128, KC * E], F32, name="wrsb")
        nc.sync.dma_start(wr_sb[:], wr[:])
        br_sb = cpool.tile([128, E], F32, name="brsb")
        nc.sync.dma_start(br_sb[:], brt[:])

        xf_pool = ctx.enter_context(tc.tile_pool(name="xf", bufs=2 * KC))
        g_pool = ctx.enter_context(tc.tile_pool(name="gate8", bufs=3 * NSUB))
        s_pool = ctx.enter_context(tc.tile_pool(name="gate1", bufs=4 * NSUB))
        pr_pool = ctx.enter_context(tc.tile_pool(name="pr", bufs=4, space="PSUM"))

        prw = pr_pool.tile([128, E], F32, name="prr")
        nc.tensor.matmul(prw[0:E, :], wr_sb[:, 0:E], wr_sb[:, 0:E], start=True, stop=True)

        for g in range(RNG):
            t0 = g * GT
            xft = []
            for k in range(KC):
                xt = xf_pool.tile([128, GT], F32, name="xft")
                nc.sync.dma_start(xt[:], xf[k, :, t0 : t0 + GT])
                xft.append(xt)
            prs = []
            for t in range(NSUB):
                ts128 = slice(t * 128, (t + 1) * 128)
                pr = pr_pool.tile([128, E], F32, name="prr")
                for k in range(KC):
                    nc.tensor.matmul(
                        pr[:],
                        xft[k][:, ts128],
                        wr_sb[:, k * E : (k + 1) * E],
                        start=(k == 0),
                        stop=(k == KC - 1),
                    )
                prs.append(pr)
            # Batched top-2 gate math for the whole group: all elementwise
            # ops run once on [128, NSUB*E] tiles (viewed [128, NSUB, E])
            # instead of NSUB separate chains of tiny [128, E] ops.
            NE = NSUB * E
            sh3 = (128, NSUB, E)
            u = g_pool.tile([128, NE], F32, name="u")
            for t in range(NSUB):
                nc.scalar.copy(u[:, t * E : (t + 1) * E], prs[t][:])
            uv = u[:].rearrange("p (t e) -> p t e", e=E)
            nc.vector.tensor_tensor(
                uv, uv, br_sb[:, None, :].broadcast_to(sh3), op=ALU.add
            )
            m1n = s_pool.tile([128, NSUB], F32, name="m1n")
            nc.vector.tensor_reduce(m1n[:], uv, axis=AX.X, op=ALU.max, negate=True)
            # u := logits - m1  (<= 0, == 0 at the top-1 entry)
            nc.vector.tensor_tensor(
                uv, uv, m1n[:, :, None].broadcast_to(sh3), op=ALU.add
            )
            mask1 = g_pool.tile([128, NE], F32, name="mask1")
            nc.vector.tensor_scalar(mask1[:], u[:], 0.0, None, ALU.is_ge)
            u2 = g_pool.tile([128, NE], F32, name="u2")
            nc.vector.tensor_scalar(u2[:], mask1[:], -1.0e30, None, ALU.mult)
            nc.vector.tensor_add(u2[:], u2[:], u[:])
            m2n = s_pool.tile([128, NSUB], F32, name="m2n")
            nc.vector.tensor_reduce(
                m2n[:],
                u2[:].rearrange("p (t e) -> p t e", e=E),
                axis=AX.X,
                op=ALU.max,
                negate=True,
            )
            z = g_pool.tile([128, NE], F32, name="z")
            nc.scalar.activation(z[:], u[:], AF.Exp)
            # mask2: u >= (l2nd - m1)  <=>  u + m2n >= 0
            mask2 = g_pool.tile([128, NE], F32, name="mask2")
            nc.vector.tensor_tensor(
                mask2[:].rearrange("p (t e) -> p t e", e=E),
                uv,
                m2n[:, :, None].broadcast_to(sh3),
                op=ALU.add,
            )
            nc.vector.tensor_scalar(mask2[:], mask2[:], 0.0, None, ALU.is_ge)
            zs = g_pool.tile([128, NE], F32, name="zs")
            nc.vector.tensor_mul(zs[:], z[:], mask2[:])
            den = s_pool.tile([128, NSUB], F32, name="den")
            nc.vector.tensor_reduce(
                den[:],
                zs[:].rearrange("p (t e) -> p t e", e=E),
                axis=AX.X,
                op=ALU.add,
            )
            rden = s_pool.tile([128, NSUB], F32, name="rden")
            nc.vector.reciprocal(rden[:], den[:])
            gate = g_pool.tile([128, NE], F32, name="gatet")
            nc.vector.tensor_tensor(
                gate[:].rearrange("p (t e) -> p t e", e=E),
                zs[:].rearrange("p (t e) -> p t e", e=E),
                rden[:, :, None].broadcast_to(sh3),
                op=ALU.mult,
            )
            nc.sync.dma_start(
                gateo[t0 : t0 + GT, :].rearrange("(t p) e -> p t e", p=128),
                gate[:].rearrange("p (t e) -> p t e", e=E),
            )
    _split_multi_waits(nc)
    return nc


def _build_ffn_nc(groups) -> bass.Bass:
    """Phase B: bf16 FFN over sum(groups) gathered tokens, gate-scaled."""
    CAP = sum(groups)
    nc = bass.Bass()
    xb = nc.declare_dram_parameter("xb", [KC, 128, CAP], BF, isOutput=False)
    w1 = nc.declare_dram_parameter("w1", [128, MC, KC * 128], BF, isOutput=False)
    w2 = nc.declare_dram_parameter("w2", [128, MC * D], BF, isOutput=False)
    b1 = nc.declare_dram_parameter("b1", [128, MC], F32, isOutput=False)
    gv = nc.declare_dram_parameter("gv", [CAP], F32, isOutput=False)
    outy = nc.declare_dram_parameter("outy", [CAP, D], F32, isOutput=True)

    with ExitStack() as ctx:
        tc = ctx.enter_context(tile.TileContext(nc))
        cpool = ctx.enter_context(tc.tile_pool(name="const", bufs=1))
        # W2 is resident in SBUF but not needed until MM2 of group 0
        # (~100us in); its chunk DMAs are emitted inside group 0's MM1 loop
        # below so the 8MB load doesn't queue ahead of the group-0 xb/w1
        # tiles the very first matmul waits on (measured 30us+ PE start
        # bubble when loaded up front).
        w2_sb = cpool.tile([128, MC * D], BF, name="w2sb")
        b1_sb = cpool.tile([128, MC], F32, name="b1sb")
        nc.sync.dma_start(b1_sb[:], b1[:])
        g_sb = cpool.tile([128, CAP // 128], F32, name="gsb")
        # gv[CAP] -> [128, CAP/128] with token t at [t%128, t//128]
        nc.sync.dma_start(
            g_sb[:], gv.rearrange("(s p) -> p s", p=128)
        )

        xb_pool = ctx.enter_context(tc.tile_pool(name="xbt", bufs=3 * KC))
        w1_pool = ctx.enter_context(tc.tile_pool(name="w1t", bufs=24))
        h_pool = ctx.enter_context(tc.tile_pool(name="ht", bufs=MC + 2))
        y_pool = ctx.enter_context(tc.tile_pool(name="yt", bufs=4))
        ph_pool = ctx.enter_context(tc.tile_pool(name="ph", bufs=4, space="PSUM"))
        py_pool = ctx.enter_context(tc.tile_pool(name="py", bufs=4, space="PSUM"))

        # first PE instruction depends on one DMA queue only
        ph0 = ph_pool.tile([128, GT], F32, name="ph")
        nc.tensor.matmul(
            ph0[0:MC, 0:MC], b1_sb[:, 0:MC], b1_sb[:, 0:MC], start=True, stop=True
        )

        t0 = 0
        for gt in groups:
            xbt = []
            for k in range(KC):
                xt = xb_pool.tile([128, gt], BF, name="xbt")
                nc.sync.dma_start(xt[:], xb[k, :, t0 : t0 + gt])
                xbt.append(xt)
            hts = []
            for m in range(MC):
                w1t = w1_pool.tile([128, KC * 128], BF, name="w1t")
                nc.sync.dma_start(w1t[:], w1[:, m, :])
                if t0 == 0:
                    # stream the resident W2 in behind the w1 tiles of group 0
                    nc.sync.dma_start(
                        w2_sb[:, m * D : (m + 1) * D], w2[:, m * D : (m + 1) * D]
                    )
                ph = ph_pool.tile([128, gt], F32, name="ph")
                for k in range(KC):
                    nc.tensor.matmul(
                        ph[:],
                        w1t[:, k * 128 : (k + 1) * 128],
                        xbt[k][:],
                        start=(k == 0),
                        stop=(k == KC - 1),
                    )
                ht = h_pool.tile([128, gt], BF, name="ht")
                nc.scalar.activation(ht[:], ph[:], AF.Relu, bias=b1_sb[:, m : m + 1])
                hts.append(ht)
            for t in range(gt // 128):
                ts128 = slice(t * 128, (t + 1) * 128)
                gcol = g_sb[:, (t0 // 128) + t : (t0 // 128) + t + 1]
                for dh in range(2):
                    py = py_pool.tile([128, 512], F32, name="py")
                    for m in range(MC):
                        nc.tensor.matmul(
                            py[:],
                            hts[m][:, ts128],
                            w2_sb[:, m * D + dh * 512 : m * D + (dh + 1) * 512],
                            start=(m == 0),
                            stop=(m == MC - 1),
                        )
                    yt = y_pool.tile([128, 512], F32, name="yt")
                    nc.scalar.mul(yt[:], py[:], gcol)
                    nc.sync.dma_start(
                        outy[
                            t0 + t * 128 : t0 + (t + 1) * 128,
                            dh * 512 : (dh + 1) * 512,
                        ],
                        yt[:],
                    )
            t0 += gt
    _split_multi_waits(nc)
    return nc


def _get_ffn(cap_needed: int):
    """FFN NEFF with capacity >= cap_needed (cached; grows on demand)."""
    cap = max(DEFAULT_CAP, ((cap_needed + 127) // 128) * 128)
    if _CACHE.get("fcap", 0) < cap:
        n512, rem = divmod(cap, 512)
        groups = [512] * n512 + ([rem] if rem else [])
        _CACHE["fnc"] = _build_ffn_nc(groups)
        _CACHE["fcap"] = cap
    return _CACHE["fnc"], _CACHE["fcap"]


def run_on_device(x, Wr, br, W1, b1, W2, b2, trace=False):
    x = np.asarray(x, np.float32).reshape(NTOK, D)
    if "rnc" not in _CACHE:
        _CACHE["rnc"] = _build_router_nc()
    rnc = _CACHE["rnc"]

    # ---- Phase A: router, token-sharded over cores -----------------------
    wrc = np.ascontiguousarray(
        np.asarray(Wr, np.float32).reshape(KC, 128, E).transpose(1, 0, 2).reshape(128, KC * E)
    )
    brc = np.ascontiguousarray(np.broadcast_to(np.asarray(br, np.float32), (128, E)))
    in_maps_a = []
    for c in range(E):
        xs = x[c * RTOK : (c + 1) * RTOK]
        xfa = np.ascontiguousarray(xs.reshape(RTOK, KC, 128).transpose(1, 2, 0))
        in_maps_a.append({"xf": xfa, "wr": wrc, "brt": brc})
    res_a = run_bass_kernel_spmd(rnc, in_maps_a, core_ids=list(range(E)), trace=trace)
    gate = np.concatenate([r["gateo"] for r in res_a.results], axis=0)  # [NTOK, E]

    # ---- Host dispatch: gather per-expert token rows ---------------------
    xb16 = x.astype(BF16)
    idxs = [np.nonzero(gate[:, e] > 0.0)[0] for e in range(E)]
    fnc, CAP = _get_ffn(max(len(i) for i in idxs))
    in_maps_b = []
    for e in range(E):
        idx = idxs[e]
        xg = np.zeros((CAP, D), BF16)
        xg[: len(idx)] = xb16[idx]
        gvv = np.zeros((CAP,), np.float32)
        gvv[: len(idx)] = gate[idx, e]
        w1c = np.ascontiguousarray(
            np.asarray(W1[e], np.float32)
            .reshape(KC, 128, MC, 128)
            .transpose(1, 2, 0, 3)
            .reshape(128, MC, KC * 128)
        ).astype(BF16)
        w2c = np.ascontiguousarray(
            np.asarray(W2[e], np.float32).reshape(MC, 128, D).transpose(1, 0, 2).reshape(128, MC * D)
        ).astype(BF16)
        b1c = np.ascontiguousarray(np.asarray(b1[e], np.float32).reshape(MC, 128).T)
        in_maps_b.append(
            {
                "xb": np.ascontiguousarray(xg.reshape(CAP, KC, 128).transpose(1, 2, 0)),
                "w1": w1c,
                "w2": w2c,
                "b1": b1c,
                "gv": gvv,
            }
        )
    res_b = run_bass_kernel_spmd(fnc, in_maps_b, core_ids=list(range(E)), trace=trace)

    # ---- Host combine: scatter-add the two expert partials per token -----
    out = gate.astype(np.float32) @ np.asarray(b2, np.float32)
    for e in range(E):
        idx = idxs[e]
        out[idx] += res_b.results[e]["outy"][: len(idx)]
    return out.reshape(B, S, D), (res_a, res_b)


def kernel(x, Wr, br, W1, b1, W2, b2):
    out, _ = run_on_device(x, Wr, br, W1, b1, W2, b2, trace=False)
    return out


# revision 15
# speedup vs baseline: 1.0027x; 1.0027x over previous
"""Sparse MoE kernel: device router + host dispatch + device per-expert FFN.

Phase A (device, SPMD over 8 cores, token-sharded): each core computes the
fp32 router (logits -> top-2 renormalized dense gate) for its 1024-token
shard. All routing FLOPs are on device.

Host dispatch: membership is already encoded in the gate (gate[t,e] > 0 iff
expert e is in token t's top-2), so the host only gathers each expert's
token rows of x (data movement, no routing math) and pads to a fixed
capacity (default 2176; actual per-expert counts are 1973..2151; zero-pad
tokens carry gate 0 and contribute nothing; the FFN NEFF is rebuilt larger
at runtime if an expert ever exceeds the built capacity).

Phase B (device, SPMD, expert-parallel): core e runs the FFN on its C
gathered tokens in bf16 with fp32 accumulation and scales by its gate
column. Host scatter-adds the two expert contributions per token and adds
the gate-weighted b2 term.
"""

import sys

for _p in ("/opt/trn_rl_repo",):
    if _p not in sys.path:
        sys.path.insert(0, _p)

from contextlib import ExitStack

import ml_dtypes
import numpy as np

import concourse.bass as bass
import concourse.mybir as mybir
import concourse.tile as tile
from concourse.bass_utils import run_bass_kernel_spmd

BF16 = ml_dtypes.bfloat16
F32 = mybir.dt.float32
BF = mybir.dt.bfloat16
AF = mybir.ActivationFunctionType
ALU = mybir.AluOpType
AX = mybir.AxisListType

B, S, D, H, E = 4, 2048, 1024, 4096, 8
NTOK = B * S
GT = 512
KC = D // 128
MC = H // 128
NSUB = GT // 128
RTOK = NTOK // E          # tokens per core in phase A (1024)
RNG = RTOK // GT          # 2 groups
# Default per-expert token capacity in phase B. Actual per-expert top-2 loads
# on this problem's fixed inputs are 1973..2151 (mean 2048). If an expert ever
# receives more than the built capacity, the FFN NEFF is rebuilt larger at
# runtime (see _get_ffn).
DEFAULT_CAP = 2176

_CACHE: dict = {}


_SPLIT_SKIP: set = set()


def _split_multi_waits(nc: bass.Bass) -> None:
    """Walrus allows one sync-wait command per instruction; split extras
    onto same-engine InstNoOps (same mechanism Tile uses for drains)."""
    for blk in nc.m.functions[0].blocks:
        insts = blk.instructions
        idx = 0
        while idx < len(insts):
            i = insts[idx]
            si = i.sync_info
            if (
                si is not None
                and len(si.on_wait) >= 2
                and type(i).__name__ not in _SPLIT_SKIP
            ):
                waits = list(si.on_wait)
                for w in waits[:-1]:
                    nop = mybir.InstNoOp(
                        name=nc.get_next_instruction_name(),
                        sync_info=mybir.SyncInfo(on_wait=[w], on_update=[]),
                        bass_nofuse=True,
                        engine=i.engine,
                    )
                    insts.insert(idx, nop)
                    idx += 1
                si.on_wait = [waits[-1]]
            idx += 1


def _emit_gate(nc, tc, pools, pr, br_sb, gateo_slice):
    """Top-2 renormalized gate from fp32 logits psum pr [128, E].
    Returns the [128, E] gate tile (also DMA'd to gateo_slice)."""
    g_pool, s_pool = pools
    logits = g_pool.tile([128, E], F32, name="logits")
    nc.vector.tensor_add(logits[:], pr[:], br_sb[:])
    m1n = s_pool.tile([128, 1], F32, name="m1n")
    nc.vector.tensor_reduce(m1n[:], logits[:], axis=AX.X, op=ALU.max, negate=True)
    mask1 = g_pool.tile([128, E], F32, name="mask1")
    nc.vector.tensor_scalar(mask1[:], logits[:], m1n[:, 0:1], 0.0, ALU.add, ALU.is_ge)
    l2 = g_pool.tile([128, E], F32, name="l2")
    nc.vector.tensor_scalar(l2[:], mask1[:], -1.0e30, None, ALU.mult)
    nc.vector.tensor_add(l2[:], l2[:], logits[:])
    m2n = s_pool.tile([128, 1], F32, name="m2n")
    nc.vector.tensor_reduce(m2n[:], l2[:], axis=AX.X, op=ALU.max, negate=True)
    z = g_pool.tile([128, E], F32, name="z")
    nc.scalar.activation(z[:], logits[:], AF.Exp, bias=m1n[:, 0:1])
    mask2 = g_pool.tile([128, E], F32, name="mask2")
    nc.vector.tensor_scalar(mask2[:], logits[:], m2n[:, 0:1], 0.0, ALU.add, ALU.is_ge)
    zs = g_pool.tile([128, E], F32, name="zs")
    nc.vector.tensor_mul(zs[:], z[:], mask2[:])
    den = s_pool.tile([128, 1], F32, name="den")
    nc.vector.tensor_reduce(den[:], zs[:], axis=AX.X, op=ALU.add)
    rden = s_pool.tile([128, 1], F32, name="rden")
    nc.vector.reciprocal(rden[:], den[:])
    gate = g_pool.tile([128, E], F32, name="gatet")
    nc.vector.tensor_scalar(gate[:], zs[:], rden[:, 0:1], None, ALU.mult)
    nc.sync.dma_start(gateo_slice, gate[:])
    return gate


def _build_router_nc() -> bass.Bass:
    """Phase A: fp32 router over this core's RTOK-token shard."""
    nc = bass.Bass()
    xf = nc.declare_dram_parameter("xf", [KC, 128, RTOK], F32, isOutput=False)
    wr = nc.declare_dram_parameter("wr", [128, KC * E], F32, isOutput=False)
    brt = nc.declare_dram_parameter("brt", [128, E], F32, isOutput=False)
    gateo = nc.declare_dram_parameter("gateo", [RTOK, E], F32, isOutput=True)

    with ExitStack() as ctx:
        tc = ctx.enter_context(tile.TileContext(nc))
        cpool = ctx.enter_context(tc.tile_pool(name="const", bufs=1))
        wr_sb = cpool.tile([128, KC * E], F32, name="wrsb")
        nc.sync.dma_start(wr_sb[:], wr[:])
        br_sb = cpool.tile([128, E], F32, name="brsb")
        nc.sync.dma_start(br_sb[:], brt[:])

        xf_pool = ctx.enter_context(tc.tile_pool(name="xf", bufs=2 * KC))
        g_pool = ctx.enter_context(tc.tile_pool(name="gate8", bufs=3 * NSUB))
        s_pool = ctx.enter_context(tc.tile_pool(name="gate1", bufs=4 * NSUB))
        pr_pool = ctx.enter_context(tc.tile_pool(name="pr", bufs=4, space="PSUM"))

        prw = pr_pool.tile([128, E], F32, name="prr")
        nc.tensor.matmul(prw[0:E, :], wr_sb[:, 0:E], wr_sb[:, 0:E], start=True, stop=True)

        for g in range(RNG):
            t0 = g * GT
            xft = []
            for k in range(KC):
                xt = xf_pool.tile([128, GT], F32, name="xft")
                nc.sync.dma_start(xt[:], xf[k, :, t0 : t0 + GT])
                xft.append(xt)
            prs = []
            for t in range(NSUB):
                ts128 = slice(t * 128, (t + 1) * 128)
                pr = pr_pool.tile([128, E], F32, name="prr")
                for k in range(KC):
                    nc.tensor.matmul(
                        pr[:],
                        xft[k][:, ts128],
                        wr_sb[:, k * E : (k + 1) * E],
                        start=(k == 0),
                        stop=(k == KC - 1),
                    )
                prs.append(pr)
            # Batched top-2 gate math for the whole group: all elementwise
            # ops run once on [128, NSUB*E] tiles (viewed [128, NSUB, E])
            # instead of NSUB separate chains of tiny [128, E] ops.
            NE = NSUB * E
            sh3 = (128, NSUB, E)
            u = g_pool.tile([128, NE], F32, name="u")
            for t in range(NSUB):
                nc.scalar.copy(u[:, t * E : (t + 1) * E], prs[t][:])
            uv = u[:].rearrange("p (t e) -> p t e", e=E)
            nc.vector.tensor_tensor(
                uv, uv, br_sb[:, None, :].broadcast_to(sh3), op=ALU.add
            )
            m1n = s_pool.tile([128, NSUB], F32, name="m1n")
            nc.vector.tensor_reduce(m1n[:], uv, axis=AX.X, op=ALU.max, negate=True)
            # u := logits - m1  (<= 0, == 0 at the top-1 entry)
            nc.vector.tensor_tensor(
                uv, uv, m1n[:, :, None].broadcast_to(sh3), op=ALU.add
            )
            mask1 = g_pool.tile([128, NE], F32, name="mask1")
            nc.vector.tensor_scalar(mask1[:], u[:], 0.0, None, ALU.is_ge)
            u2 = g_pool.tile([128, NE], F32, name="u2")
            nc.vector.tensor_scalar(u2[:], mask1[:], -1.0e30, None, ALU.mult)
            nc.vector.tensor_add(u2[:], u2[:], u[:])
            m2n = s_pool.tile([128, NSUB], F32, name="m2n")
            nc.vector.tensor_reduce(
                m2n[:],
                u2[:].rearrange("p (t e) -> p t e", e=E),
                axis=AX.X,
                op=ALU.max,
                negate=True,
            )
            z = g_pool.tile([128, NE], F32, name="z")
            nc.scalar.activation(z[:], u[:], AF.Exp)
            # mask2: u >= (l2nd - m1)  <=>  u + m2n >= 0
            mask2 = g_pool.tile([128, NE], F32, name="mask2")
            nc.vector.tensor_tensor(
                mask2[:].rearrange("p (t e) -> p t e", e=E),
                uv,
                m2n[:, :, None].broadcast_to(sh3),
                op=ALU.add,
            )
            nc.vector.tensor_scalar(mask2[:], mask2[:], 0.0, None, ALU.is_ge)
            zs = g_pool.tile([128, NE], F32, name="zs")
            nc.vector.tensor_mul(zs[:], z[:], mask2[:])
            den = s_pool.tile([128, NSUB], F32, name="den")
            nc.vector.tensor_reduce(
                den[:],
                zs[:].rearrange("p (t e) -> p t e", e=E),
                axis=AX.X,
                op=ALU.add,
            )
            rden = s_pool.tile([128, NSUB], F32, name="rden")
            nc.vector.reciprocal(rden[:], den[:])
            gate = g_pool.tile([128, NE], F32, name="gatet")
            nc.vector.tensor_tensor(
                gate[:].rearrange("p (t e) -> p t e", e=E),
                zs[:].rearrange("p (t e) -> p t e", e=E),
                rden[:, :, None].broadcast_to(sh3),
                op=ALU.mult,
            )
            nc.sync.dma_start(
                gateo[t0 : t0 + GT, :].rearrange("(t p) e -> p t e", p=128),
                gate[:].rearrange("p (t e) -> p t e", e=E),
            )
    _split_multi_waits(nc)
    return nc


def _build_ffn_nc(groups) -> bass.Bass:
    """Phase B: bf16 FFN over sum(groups) gathered tokens, gate-scaled."""
    CAP = sum(groups)
    nc = bass.Bass()
    xb = nc.declare_dram_parameter("xb", [KC, 128, CAP], BF, isOutput=False)
    w1 = nc.declare_dram_parameter("w1", [128, MC, KC * 128], BF, isOutput=False)
    w2 = nc.declare_dram_parameter("w2", [128, MC * D], BF, isOutput=False)
    b1 = nc.declare_dram_parameter("b1", [128, MC], F32, isOutput=False)
    gv = nc.declare_dram_parameter("gv", [CAP], F32, isOutput=False)
    outy = nc.declare_dram_parameter("outy", [CAP, D], F32, isOutput=True)

    with ExitStack() as ctx:
        tc = ctx.enter_context(tile.TileContext(nc))
        cpool = ctx.enter_context(tc.tile_pool(name="const", bufs=1))
        # W2 is resident in SBUF but not needed until MM2 of group 0
        # (~100us in); its chunk DMAs are emitted inside group 0's MM1 loop
        # below so the 8MB load doesn't queue ahead of the group-0 xb/w1
        # tiles the very first matmul waits on (measured 30us+ PE start
        # bubble when loaded up front).
        w2_sb = cpool.tile([128, MC * D], BF, name="w2sb")
        w1_sb = cpool.tile([128, MC * KC * 128], BF, name="w1sb")
        b1_sb = cpool.tile([128, MC], F32, name="b1sb")
        nc.sync.dma_start(b1_sb[:], b1[:])
        g_sb = cpool.tile([128, CAP // 128], F32, name="gsb")
        # gv[CAP] -> [128, CAP/128] with token t at [t%128, t//128]
        nc.sync.dma_start(
            g_sb[:], gv.rearrange("(s p) -> p s", p=128)
        )

        xb_pool = ctx.enter_context(tc.tile_pool(name="xbt", bufs=2 * KC))
        h_pool = ctx.enter_context(tc.tile_pool(name="ht", bufs=MC + 2))
        y_pool = ctx.enter_context(tc.tile_pool(name="yt", bufs=4))
        ph_pool = ctx.enter_context(tc.tile_pool(name="ph", bufs=4, space="PSUM"))
        py_pool = ctx.enter_context(tc.tile_pool(name="py", bufs=4, space="PSUM"))

        # first PE instruction depends on one DMA queue only
        ph0 = ph_pool.tile([128, GT], F32, name="ph")
        nc.tensor.matmul(
            ph0[0:MC, 0:MC], b1_sb[:, 0:MC], b1_sb[:, 0:MC], start=True, stop=True
        )

        t0 = 0
        for gt in groups:
            xbt = []
            for k in range(KC):
                xt = xb_pool.tile([128, gt], BF, name="xbt")
                nc.sync.dma_start(xt[:], xb[k, :, t0 : t0 + gt])
                xbt.append(xt)
            hts = []
            for m in range(MC):
                if t0 == 0:
                    # W1 and W2 are SBUF-resident; their chunk loads stream in
                    # behind group 0's compute (one 256KB chunk of each per
                    # m-iteration) so nothing queues ahead of the first
                    # matmuls and later groups do no weight DMA at all.
                    nc.sync.dma_start(
                        w1_sb[:, m * KC * 128 : (m + 1) * KC * 128], w1[:, m, :]
                    )
                    nc.sync.dma_start(
                        w2_sb[:, m * D : (m + 1) * D], w2[:, m * D : (m + 1) * D]
                    )
                ph = ph_pool.tile([128, gt], F32, name="ph")
                w1m = m * KC * 128
                for k in range(KC):
                    nc.tensor.matmul(
                        ph[:],
                        w1_sb[:, w1m + k * 128 : w1m + (k + 1) * 128],
                        xbt[k][:],
                        start=(k == 0),
                        stop=(k == KC - 1),
                    )
                ht = h_pool.tile([128, gt], BF, name="ht")
                nc.scalar.activation(ht[:], ph[:], AF.Relu, bias=b1_sb[:, m : m + 1])
                hts.append(ht)
            for t in range(gt // 128):
                ts128 = slice(t * 128, (t + 1) * 128)
                gcol = g_sb[:, (t0 // 128) + t : (t0 // 128) + t + 1]
                for dh in range(2):
                    py = py_pool.tile([128, 512], F32, name="py")
                    for m in range(MC):
                        nc.tensor.matmul(
                            py[:],
                            hts[m][:, ts128],
                            w2_sb[:, m * D + dh * 512 : m * D + (dh + 1) * 512],
                            start=(m == 0),
                            stop=(m == MC - 1),
                        )
                    yt = y_pool.tile([128, 512], F32, name="yt")
                    nc.scalar.mul(yt[:], py[:], gcol)
                    nc.sync.dma_start(
                        outy[
                            t0 + t * 128 : t0 + (t + 1) * 128,
                            dh * 512 : (dh + 1) * 512,
                        ],
                        yt[:],
                    )
            t0 += gt
    _split_multi_waits(nc)
    return nc


def _get_ffn(cap_needed: int):
    """FFN NEFF with capacity >= cap_needed (cached; grows on demand)."""
    cap = max(DEFAULT_CAP, ((cap_needed + 127) // 128) * 128)
    if _CACHE.get("fcap", 0) < cap:
        n512, rem = divmod(cap, 512)
        groups = [512] * n512 + ([rem] if rem else [])
        _CACHE["fnc"] = _build_ffn_nc(groups)
        _CACHE["fcap"] = cap
    return _CACHE["fnc"], _CACHE["fcap"]


def run_on_device(x, Wr, br, W1, b1, W2, b2, trace=False):
    x = np.asarray(x, np.float32).reshape(NTOK, D)
    if "rnc" not in _CACHE:
        _CACHE["rnc"] = _build_router_nc()
    rnc = _CACHE["rnc"]

    # ---- Phase A: router, token-sharded over cores -----------------------
    wrc = np.ascontiguousarray(
        np.asarray(Wr, np.float32).reshape(KC, 128, E).transpose(1, 0, 2).reshape(128, KC * E)
    )
    brc = np.ascontiguousarray(np.broadcast_to(np.asarray(br, np.float32), (128, E)))
    in_maps_a = []
    for c in range(E):
        xs = x[c * RTOK : (c + 1) * RTOK]
        xfa = np.ascontiguousarray(xs.reshape(RTOK, KC, 128).transpose(1, 2, 0))
        in_maps_a.append({"xf": xfa, "wr": wrc, "brt": brc})
    res_a = run_bass_kernel_spmd(rnc, in_maps_a, core_ids=list(range(E)), trace=trace)
    gate = np.concatenate([r["gateo"] for r in res_a.results], axis=0)  # [NTOK, E]

    # ---- Host dispatch: gather per-expert token rows ---------------------
    xb16 = x.astype(BF16)
    idxs = [np.nonzero(gate[:, e] > 0.0)[0] for e in range(E)]
    fnc, CAP = _get_ffn(max(len(i) for i in idxs))
    in_maps_b = []
    for e in range(E):
        idx = idxs[e]
        xg = np.zeros((CAP, D), BF16)
        xg[: len(idx)] = xb16[idx]
        gvv = np.zeros((CAP,), np.float32)
        gvv[: len(idx)] = gate[idx, e]
        w1c = np.ascontiguousarray(
            np.asarray(W1[e], np.float32)
            .reshape(KC, 128, MC, 128)
            .transpose(1, 2, 0, 3)
            .reshape(128, MC, KC * 128)
        ).astype(BF16)
        w2c = np.ascontiguousarray(
            np.asarray(W2[e], np.float32).reshape(MC, 128, D).transpose(1, 0, 2).reshape(128, MC * D)
        ).astype(BF16)
        b1c = np.ascontiguousarray(np.asarray(b1[e], np.float32).reshape(MC, 128).T)
        in_maps_b.append(
            {
                "xb": np.ascontiguousarray(xg.reshape(CAP, KC, 128).transpose(1, 2, 0)),
                "w1": w1c,
                "w2": w2c,
                "b1": b1c,
                "gv": gvv,
            }
        )
    res_b = run_bass_kernel_spmd(fnc, in_maps_b, core_ids=list(range(E)), trace=trace)

    # ---- Host combine: scatter-add the two expert partials per token -----
    out = gate.astype(np.float32) @ np.asarray(b2, np.float32)
    for e in range(E):
        idx = idxs[e]
        out[idx] += res_b.results[e]["outy"][: len(idx)]
    return out.reshape(B, S, D), (res_a, res_b)


def kernel(x, Wr, br, W1, b1, W2, b2):
    out, _ = run_on_device(x, Wr, br, W1, b1, W2, b2, trace=False)
    return out


# revision 17
# speedup vs baseline: 1.0122x; 1.0094x over previous
"""Sparse MoE kernel: device router + host dispatch + device per-expert FFN.

Phase A (device, SPMD over 8 cores, token-sharded): each core computes the
fp32 router (logits -> top-2 renormalized dense gate) for its 1024-token
shard. All routing FLOPs are on device.

Host dispatch: membership is already encoded in the gate (gate[t,e] > 0 iff
expert e is in token t's top-2), so the host only gathers each expert's
token rows of x (data movement, no routing math) and pads to a fixed
capacity (default 2176; actual per-expert counts are 1973..2151; zero-pad
tokens carry gate 0 and contribute nothing; the FFN NEFF is rebuilt larger
at runtime if an expert ever exceeds the built capacity).

Phase B (device, SPMD, expert-parallel): core e runs the FFN on its C
gathered tokens in bf16 with fp32 accumulation and scales by its gate
column. Host scatter-adds the two expert contributions per token and adds
the gate-weighted b2 term.
"""

import sys

for _p in ("/opt/trn_rl_repo",):
    if _p not in sys.path:
        sys.path.insert(0, _p)

from contextlib import ExitStack

import ml_dtypes
import numpy as np

import concourse.bass as bass
import concourse.mybir as mybir
import concourse.tile as tile
from concourse.bass_utils import run_bass_kernel_spmd

BF16 = ml_dtypes.bfloat16
F32 = mybir.dt.float32
BF = mybir.dt.bfloat16
AF = mybir.ActivationFunctionType
ALU = mybir.AluOpType
AX = mybir.AxisListType

B, S, D, H, E = 4, 2048, 1024, 4096, 8
NTOK = B * S
GT = 512
KC = D // 128
MC = H // 128
NSUB = GT // 128
RTOK = NTOK // E          # tokens per core in phase A (1024)
RNG = RTOK // GT          # 2 groups
# Default per-expert token capacity in phase B. Actual per-expert top-2 loads
# on this problem's fixed inputs are 1973..2151 (mean 2048). If an expert ever
# receives more than the built capacity, the FFN NEFF is rebuilt larger at
# runtime (see _get_ffn).
DEFAULT_CAP = 2176

_CACHE: dict = {}


_SPLIT_SKIP: set = set()


def _split_multi_waits(nc: bass.Bass) -> None:
    """Walrus allows one sync-wait command per instruction; split extras
    onto same-engine InstNoOps (same mechanism Tile uses for drains)."""
    for blk in nc.m.functions[0].blocks:
        insts = blk.instructions
        idx = 0
        while idx < len(insts):
            i = insts[idx]
            si = i.sync_info
            if (
                si is not None
                and len(si.on_wait) >= 2
                and type(i).__name__ not in _SPLIT_SKIP
            ):
                waits = list(si.on_wait)
                for w in waits[:-1]:
                    nop = mybir.InstNoOp(
                        name=nc.get_next_instruction_name(),
                        sync_info=mybir.SyncInfo(on_wait=[w], on_update=[]),
                        bass_nofuse=True,
                        engine=i.engine,
                    )
                    insts.insert(idx, nop)
                    idx += 1
                si.on_wait = [waits[-1]]
            idx += 1


def _emit_gate(nc, tc, pools, pr, br_sb, gateo_slice):
    """Top-2 renormalized gate from fp32 logits psum pr [128, E].
    Returns the [128, E] gate tile (also DMA'd to gateo_slice)."""
    g_pool, s_pool = pools
    logits = g_pool.tile([128, E], F32, name="logits")
    nc.vector.tensor_add(logits[:], pr[:], br_sb[:])
    m1n = s_pool.tile([128, 1], F32, name="m1n")
    nc.vector.tensor_reduce(m1n[:], logits[:], axis=AX.X, op=ALU.max, negate=True)
    mask1 = g_pool.tile([128, E], F32, name="mask1")
    nc.vector.tensor_scalar(mask1[:], logits[:], m1n[:, 0:1], 0.0, ALU.add, ALU.is_ge)
    l2 = g_pool.tile([128, E], F32, name="l2")
    nc.vector.tensor_scalar(l2[:], mask1[:], -1.0e30, None, ALU.mult)
    nc.vector.tensor_add(l2[:], l2[:], logits[:])
    m2n = s_pool.tile([128, 1], F32, name="m2n")
    nc.vector.tensor_reduce(m2n[:], l2[:], axis=AX.X, op=ALU.max, negate=True)
    z = g_pool.tile([128, E], F32, name="z")
    nc.scalar.activation(z[:], logits[:], AF.Exp, bias=m1n[:, 0:1])
    mask2 = g_pool.tile([128, E], F32, name="mask2")
    nc.vector.tensor_scalar(mask2[:], logits[:], m2n[:, 0:1], 0.0, ALU.add, ALU.is_ge)
    zs = g_pool.tile([128, E], F32, name="zs")
    nc.vector.tensor_mul(zs[:], z[:], mask2[:])
    den = s_pool.tile([128, 1], F32, name="den")
    nc.vector.tensor_reduce(den[:], zs[:], axis=AX.X, op=ALU.add)
    rden = s_pool.tile([128, 1], F32, name="rden")
    nc.vector.reciprocal(rden[:], den[:])
    gate = g_pool.tile([128, E], F32, name="gatet")
    nc.vector.tensor_scalar(gate[:], zs[:], rden[:, 0:1], None, ALU.mult)
    nc.sync.dma_start(gateo_slice, gate[:])
    return gate


def _build_router_nc() -> bass.Bass:
    """Phase A: fp32 router over this core's RTOK-token shard."""
    nc = bass.Bass()
    xf = nc.declare_dram_parameter("xf", [KC, 128, RTOK], F32, isOutput=False)
    wr = nc.declare_dram_parameter("wr", [128, KC * E], F32, isOutput=False)
    brt = nc.declare_dram_parameter("brt", [128, E], F32, isOutput=False)
    iden = nc.declare_dram_parameter("iden", [E, E], F32, isOutput=False)
    gateo = nc.declare_dram_parameter("gateo", [RTOK, E], F32, isOutput=True)

    with ExitStack() as ctx:
        tc = ctx.enter_context(tile.TileContext(nc))
        cpool = ctx.enter_context(tc.tile_pool(name="const", bufs=1))
        wr_sb = cpool.tile([128, KC * E], F32, name="wrsb")
        nc.sync.dma_start(wr_sb[:], wr[:])
        br_sb = cpool.tile([128, E], F32, name="brsb")
        nc.sync.dma_start(br_sb[:], brt[:])
        id_sb = cpool.tile([E, E], F32, name="idsb")
        nc.sync.dma_start(id_sb[:], iden[:])

        xf_pool = ctx.enter_context(tc.tile_pool(name="xf", bufs=2 * KC))
        g_pool = ctx.enter_context(tc.tile_pool(name="gate8", bufs=3 * NSUB))
        s_pool = ctx.enter_context(tc.tile_pool(name="gate1", bufs=4 * NSUB))
        pr_pool = ctx.enter_context(tc.tile_pool(name="pr", bufs=4, space="PSUM"))
        pl_pool = ctx.enter_context(tc.tile_pool(name="pl", bufs=2, space="PSUM"))
        sl_pool = ctx.enter_context(tc.tile_pool(name="sl", bufs=2))

        prw = pr_pool.tile([128, E], F32, name="prr")
        nc.tensor.matmul(prw[0:E, :], wr_sb[:, 0:E], wr_sb[:, 0:E], start=True, stop=True)

        for g in range(RNG):
            t0 = g * GT
            xft = []
            for k in range(KC):
                xt = xf_pool.tile([128, GT], F32, name="xft")
                nc.sync.dma_start(xt[:], xf[k, :, t0 : t0 + GT])
                xft.append(xt)
            pl = pl_pool.tile([E, GT], F32, name="pl")
            for k in range(KC):
                nc.tensor.matmul(
                    pl[:],
                    wr_sb[:, k * E : (k + 1) * E],
                    xft[k][:],
                    start=(k == 0),
                    stop=(k == KC - 1),
                )
            sl = sl_pool.tile([E, GT], F32, name="sl")
            nc.scalar.copy(sl[:], pl[:])
            prs = []
            for t in range(NSUB):
                pr = pr_pool.tile([128, E], F32, name="prr")
                nc.tensor.transpose(pr[:], sl[:, t * 128 : (t + 1) * 128], id_sb[:])
                prs.append(pr)
            # Batched top-2 gate math for the whole group: all elementwise
            # ops run once on [128, NSUB*E] tiles (viewed [128, NSUB, E])
            # instead of NSUB separate chains of tiny [128, E] ops.
            NE = NSUB * E
            sh3 = (128, NSUB, E)
            u = g_pool.tile([128, NE], F32, name="u")
            for t in range(NSUB):
                nc.scalar.copy(u[:, t * E : (t + 1) * E], prs[t][:])
            uv = u[:].rearrange("p (t e) -> p t e", e=E)
            nc.vector.tensor_tensor(
                uv, uv, br_sb[:, None, :].broadcast_to(sh3), op=ALU.add
            )
            m1n = s_pool.tile([128, NSUB], F32, name="m1n")
            nc.vector.tensor_reduce(m1n[:], uv, axis=AX.X, op=ALU.max, negate=True)
            # u := logits - m1  (<= 0, == 0 at the top-1 entry)
            nc.vector.tensor_tensor(
                uv, uv, m1n[:, :, None].broadcast_to(sh3), op=ALU.add
            )
            mask1 = g_pool.tile([128, NE], F32, name="mask1")
            nc.vector.tensor_scalar(mask1[:], u[:], 0.0, None, ALU.is_ge)
            u2 = g_pool.tile([128, NE], F32, name="u2")
            nc.vector.tensor_scalar(u2[:], mask1[:], -1.0e30, None, ALU.mult)
            nc.vector.tensor_add(u2[:], u2[:], u[:])
            m2n = s_pool.tile([128, NSUB], F32, name="m2n")
            nc.vector.tensor_reduce(
                m2n[:],
                u2[:].rearrange("p (t e) -> p t e", e=E),
                axis=AX.X,
                op=ALU.max,
                negate=True,
            )
            z = g_pool.tile([128, NE], F32, name="z")
            nc.scalar.activation(z[:], u[:], AF.Exp)
            # mask2: u >= (l2nd - m1)  <=>  u + m2n >= 0
            mask2 = g_pool.tile([128, NE], F32, name="mask2")
            nc.vector.tensor_tensor(
                mask2[:].rearrange("p (t e) -> p t e", e=E),
                uv,
                m2n[:, :, None].broadcast_to(sh3),
                op=ALU.add,
            )
            nc.vector.tensor_scalar(mask2[:], mask2[:], 0.0, None, ALU.is_ge)
            zs = g_pool.tile([128, NE], F32, name="zs")
            nc.vector.tensor_mul(zs[:], z[:], mask2[:])
            den = s_pool.tile([128, NSUB], F32, name="den")
            nc.vector.tensor_reduce(
                den[:],
                zs[:].rearrange("p (t e) -> p t e", e=E),
                axis=AX.X,
                op=ALU.add,
            )
            rden = s_pool.tile([128, NSUB], F32, name="rden")
            nc.vector.reciprocal(rden[:], den[:])
            gate = g_pool.tile([128, NE], F32, name="gatet")
            nc.vector.tensor_tensor(
                gate[:].rearrange("p (t e) -> p t e", e=E),
                zs[:].rearrange("p (t e) -> p t e", e=E),
                rden[:, :, None].broadcast_to(sh3),
                op=ALU.mult,
            )
            nc.sync.dma_start(
                gateo[t0 : t0 + GT, :].rearrange("(t p) e -> p t e", p=128),
                gate[:].rearrange("p (t e) -> p t e", e=E),
            )
    _split_multi_waits(nc)
    return nc


def _build_ffn_nc(groups) -> bass.Bass:
    """Phase B: bf16 FFN over sum(groups) gathered tokens, gate-scaled."""
    CAP = sum(groups)
    nc = bass.Bass()
    xb = nc.declare_dram_parameter("xb", [KC, 128, CAP], BF, isOutput=False)
    w1 = nc.declare_dram_parameter("w1", [128, MC, KC * 128], BF, isOutput=False)
    w2 = nc.declare_dram_parameter("w2", [128, MC * D], BF, isOutput=False)
    b1 = nc.declare_dram_parameter("b1", [128, MC], F32, isOutput=False)
    gv = nc.declare_dram_parameter("gv", [CAP], F32, isOutput=False)
    outy = nc.declare_dram_parameter("outy", [CAP, D], F32, isOutput=True)

    with ExitStack() as ctx:
        tc = ctx.enter_context(tile.TileContext(nc))
        cpool = ctx.enter_context(tc.tile_pool(name="const", bufs=1))
        # W2 is resident in SBUF but not needed until MM2 of group 0
        # (~100us in); its chunk DMAs are emitted inside group 0's MM1 loop
        # below so the 8MB load doesn't queue ahead of the group-0 xb/w1
        # tiles the very first matmul waits on (measured 30us+ PE start
        # bubble when loaded up front).
        w2_sb = cpool.tile([128, MC * D], BF, name="w2sb")
        w1_sb = cpool.tile([128, MC * KC * 128], BF, name="w1sb")
        b1_sb = cpool.tile([128, MC], F32, name="b1sb")
        nc.sync.dma_start(b1_sb[:], b1[:])
        g_sb = cpool.tile([128, CAP // 128], F32, name="gsb")
        # gv[CAP] -> [128, CAP/128] with token t at [t%128, t//128]
        nc.sync.dma_start(
            g_sb[:], gv.rearrange("(s p) -> p s", p=128)
        )

        xb_pool = ctx.enter_context(tc.tile_pool(name="xbt", bufs=2 * KC))
        h_pool = ctx.enter_context(tc.tile_pool(name="ht", bufs=MC + 2))
        y_pool = ctx.enter_context(tc.tile_pool(name="yt", bufs=4))
        ph_pool = ctx.enter_context(tc.tile_pool(name="ph", bufs=4, space="PSUM"))
        py_pool = ctx.enter_context(tc.tile_pool(name="py", bufs=4, space="PSUM"))

        # first PE instruction depends on one DMA queue only
        ph0 = ph_pool.tile([128, GT], F32, name="ph")
        nc.tensor.matmul(
            ph0[0:MC, 0:MC], b1_sb[:, 0:MC], b1_sb[:, 0:MC], start=True, stop=True
        )

        t0 = 0
        for gt in groups:
            xbt = []
            for k in range(KC):
                xt = xb_pool.tile([128, gt], BF, name="xbt")
                nc.sync.dma_start(xt[:], xb[k, :, t0 : t0 + gt])
                xbt.append(xt)
            hts = []
            for m in range(MC):
                if t0 == 0:
                    # W1 and W2 are SBUF-resident; their chunk loads stream in
                    # behind group 0's compute (one 256KB chunk of each per
                    # m-iteration) so nothing queues ahead of the first
                    # matmuls and later groups do no weight DMA at all.
                    nc.sync.dma_start(
                        w1_sb[:, m * KC * 128 : (m + 1) * KC * 128], w1[:, m, :]
                    )
                    nc.sync.dma_start(
                        w2_sb[:, m * D : (m + 1) * D], w2[:, m * D : (m + 1) * D]
                    )
                ph = ph_pool.tile([128, gt], F32, name="ph")
                w1m = m * KC * 128
                for k in range(KC):
                    nc.tensor.matmul(
                        ph[:],
                        w1_sb[:, w1m + k * 128 : w1m + (k + 1) * 128],
                        xbt[k][:],
                        start=(k == 0),
                        stop=(k == KC - 1),
                    )
                ht = h_pool.tile([128, gt], BF, name="ht")
                nc.scalar.activation(ht[:], ph[:], AF.Relu, bias=b1_sb[:, m : m + 1])
                hts.append(ht)
            for t in range(gt // 128):
                ts128 = slice(t * 128, (t + 1) * 128)
                gcol = g_sb[:, (t0 // 128) + t : (t0 // 128) + t + 1]
                for dh in range(2):
                    py = py_pool.tile([128, 512], F32, name="py")
                    for m in range(MC):
                        nc.tensor.matmul(
                            py[:],
                            hts[m][:, ts128],
                            w2_sb[:, m * D + dh * 512 : m * D + (dh + 1) * 512],
                            start=(m == 0),
                            stop=(m == MC - 1),
                        )
                    yt = y_pool.tile([128, 512], F32, name="yt")
                    nc.scalar.mul(yt[:], py[:], gcol)
                    nc.sync.dma_start(
                        outy[
                            t0 + t * 128 : t0 + (t + 1) * 128,
                            dh * 512 : (dh + 1) * 512,
                        ],
                        yt[:],
                    )
            t0 += gt
    _split_multi_waits(nc)
    return nc


def _get_ffn(cap_needed: int):
    """FFN NEFF with capacity >= cap_needed (cached; grows on demand)."""
    cap = max(DEFAULT_CAP, ((cap_needed + 127) // 128) * 128)
    if _CACHE.get("fcap", 0) < cap:
        n512, rem = divmod(cap, 512)
        groups = [512] * n512 + ([rem] if rem else [])
        _CACHE["fnc"] = _build_ffn_nc(groups)
        _CACHE["fcap"] = cap
    return _CACHE["fnc"], _CACHE["fcap"]


def run_on_device(x, Wr, br, W1, b1, W2, b2, trace=False):
    x = np.asarray(x, np.float32).reshape(NTOK, D)
    if "rnc" not in _CACHE:
        _CACHE["rnc"] = _build_router_nc()
    rnc = _CACHE["rnc"]

    # ---- Phase A: router, token-sharded over cores -----------------------
    wrc = np.ascontiguousarray(
        np.asarray(Wr, np.float32).reshape(KC, 128, E).transpose(1, 0, 2).reshape(128, KC * E)
    )
    brc = np.ascontiguousarray(np.broadcast_to(np.asarray(br, np.float32), (128, E)))
    in_maps_a = []
    for c in range(E):
        xs = x[c * RTOK : (c + 1) * RTOK]
        xfa = np.ascontiguousarray(xs.reshape(RTOK, KC, 128).transpose(1, 2, 0))
        in_maps_a.append({"xf": xfa, "wr": wrc, "brt": brc, "iden": np.eye(E, dtype=np.float32)})
    res_a = run_bass_kernel_spmd(rnc, in_maps_a, core_ids=list(range(E)), trace=trace)
    gate = np.concatenate([r["gateo"] for r in res_a.results], axis=0)  # [NTOK, E]

    # ---- Host dispatch: gather per-expert token rows ---------------------
    xb16 = x.astype(BF16)
    idxs = [np.nonzero(gate[:, e] > 0.0)[0] for e in range(E)]
    fnc, CAP = _get_ffn(max(len(i) for i in idxs))
    in_maps_b = []
    for e in range(E):
        idx = idxs[e]
        xg = np.zeros((CAP, D), BF16)
        xg[: len(idx)] = xb16[idx]
        gvv = np.zeros((CAP,), np.float32)
        gvv[: len(idx)] = gate[idx, e]
        w1c = np.ascontiguousarray(
            np.asarray(W1[e], np.float32)
            .reshape(KC, 128, MC, 128)
            .transpose(1, 2, 0, 3)
            .reshape(128, MC, KC * 128)
        ).astype(BF16)
        w2c = np.ascontiguousarray(
            np.asarray(W2[e], np.float32).reshape(MC, 128, D).transpose(1, 0, 2).reshape(128, MC * D)
        ).astype(BF16)
        b1c = np.ascontiguousarray(np.asarray(b1[e], np.float32).reshape(MC, 128).T)
        in_maps_b.append(
            {
                "xb": np.ascontiguousarray(xg.reshape(CAP, KC, 128).transpose(1, 2, 0)),
                "w1": w1c,
                "w2": w2c,
                "b1": b1c,
                "gv": gvv,
            }
        )
    res_b = run_bass_kernel_spmd(fnc, in_maps_b, core_ids=list(range(E)), trace=trace)

    # ---- Host combine: scatter-add the two expert partials per token -----
    out = gate.astype(np.float32) @ np.asarray(b2, np.float32)
    for e in range(E):
        idx = idxs[e]
        out[idx] += res_b.results[e]["outy"][: len(idx)]
    return out.reshape(B, S, D), (res_a, res_b)


def kernel(x, Wr, br, W1, b1, W2, b2):
    out, _ = run_on_device(x, Wr, br, W1, b1, W2, b2, trace=False)
    return out
